# revision 1
# baseline (speedup 1.0000x reference)
"""Multi-head attention (B=8, N=1024, C=768, H=12) on 8 Trainium2 NeuronCores.

Sharding: data-parallel, one batch element per core. Each core computes the
full attention block for its batch: QKV projection, per-head softmax(QK^T/8)V,
and the output projection, entirely on-chip (SBUF/PSUM).

Layout strategy (chosen so no on-device transposes are needed):
  - host passes x^T [C, N], w_qkv^T [C, 3C], w_proj^T [C, C], bias replicated
    to [128, C].
  - Q, K are produced transposed ([d, n], head-dim on partitions) by the QKV
    matmul; V is produced in natural [n, d] layout by swapping lhsT/rhs.
  - scores are computed transposed (S^T[m, n] = K Q^T) so that exp(S^T) can be
    consumed directly as the moving operand of the P@V matmul.
  - V tiles carry an appended ones-column, so the P@V matmul's 65th output row
    is the softmax denominator (row-sum of exp scores) for free.
  - normalization multiplies by a reciprocal row broadcast across partitions
    via a DRAM-bounced DMA (SBUF APs cannot partition-broadcast).

Matmul operands use dtype float32r: single-pass PE streaming (1 column/cycle,
4x faster than float32's two-pass LOW/HIGH emulation) with 11 explicit
mantissa bits. Producers round on write; DRAM inputs are pre-rounded on host.

Scheduling: attention for head pair t overlaps the remaining QKV projection
work. All PSUM users run on half-size (single-bank) accumulation groups so
the 8 banks split 2+2 (QK/V projection) + 2+2 (scores / P@V); attn_out^T
tiles reuse the SBUF slots of dead Q^T tiles so everything fits in 192KB.
"""

import sys

import numpy as np

if "/opt/trn_rl_repo" not in sys.path:
    sys.path.insert(0, "/opt/trn_rl_repo")

B = 8
N = 1024
C = 768
H = 12
D = 64
SCALE = D ** -0.5
KT = C // 128           # 6 contraction tiles over channels
MT_QK = 2 * C // 128    # 12 output tiles for Q and K (o in [0, 1536))
NT = N // 128           # 8 token tiles
PAIRS = H // 2          # 6 head pairs

_CACHE = {}


def build_program(fast=True):
    import concourse.bacc as bacc
    import concourse.mybir as mybir
    import concourse.tile as tile

    f32 = mybir.dt.float32
    f32r = mybir.dt.float32r
    Exp = mybir.ActivationFunctionType.Exp
    fm = f32r if fast else f32

    nc = bacc.Bacc("TRN2", target_bir_lowering=False, debug=False)

    xT_d = nc.dram_tensor("xT", [C, N], fm, kind="ExternalInput")
    wqkvT_d = nc.dram_tensor("wqkvT", [C, 3 * C], fm, kind="ExternalInput")
    wprojT_d = nc.dram_tensor("wprojT", [C, C], fm, kind="ExternalInput")
    bias_d = nc.dram_tensor("bias_rep", [128, C], f32, kind="ExternalInput")
    y_d = nc.dram_tensor("y", [N, C], f32, kind="ExternalOutput")

    mm = nc.tensor.matmul

    with tile.TileContext(nc) as tc:
        # qkt/aot share one 12-slot tag: each aot[t] lands in the slot of a
        # Q^T/K^T tile that died right before it (pair t's score matmuls).
        with tc.tile_pool(name="pers", bufs=1) as pers, \
             tc.tile_pool(name="qa", bufs=13) as qa, \
             tc.tile_pool(name="cyc", bufs=2) as pB, \
             tc.tile_pool(name="dramb", bufs=2, space="DRAM") as pDr, \
             tc.tile_pool(name="ps_s", bufs=3, space="PSUM") as psS, \
             tc.tile_pool(name="ps_y", bufs=2, space="PSUM") as psY:
            # Q^T,K^T tiles [d, n]: tile m holds heads 2m (parts 0:64) and
            # 2m+1 (parts 64:128); m 0..5 = Q, 6..11 = K.
            qkt = [qa.tile([128, N], fm, name=f"qkt{m}", tag="qa")
                   for m in range(MT_QK)]
            # V tiles [n-tile, pair, 130]: per pair block [V_h0 |1| V_h1 |1];
            # ones cols at 64 and 129 feed the denominator row of P@V.
            vbuf = [pers.tile([128, PAIRS, 130], fm, name=f"vbuf{i}", tag=f"vbuf{i}")
                    for i in range(NT)]

            with tc.tile_pool(name="phA", bufs=1) as pA:
                xt = [pA.tile([128, N], fm, name=f"xt{k}", tag=f"xt{k}")
                      for k in range(KT)]
                wqk = [pA.tile([128, 2 * C], fm, name=f"wqk{k}", tag=f"wqk{k}")
                       for k in range(KT)]
                wv = [pA.tile([128, C], fm, name=f"wv{k}", tag=f"wv{k}")
                      for k in range(KT)]
                for k in range(KT):
                    nc.sync.dma_start(xt[k][:], xT_d[128 * k:128 * (k + 1), :])
                for k in range(KT):
                    nc.sync.dma_start(wv[k][:],
                                      wqkvT_d[128 * k:128 * (k + 1), 2 * C:3 * C])
                for k in range(KT):
                    nc.sync.dma_start(wqk[k][:],
                                      wqkvT_d[128 * k:128 * (k + 1), 0:2 * C])
                for i in range(NT):
                    ones_ap = vbuf[i].rearrange("p a (t c) -> p a t c", c=65)[:, :, :, 64]
                    nc.vector.memset(ones_ap.bitcast(f32), 1.0)


                # ---- QKV projection, single-bank accumulation groups ----
                def emit_qk(m):
                    for j in range(2):
                        ps = psS.tile([128, 512], f32, name="qk_ps", tag="ps")
                        for k in range(KT):
                            mm(ps[:], wqk[k][:, 128 * m:128 * (m + 1)],
                               xt[k][:, 512 * j:512 * (j + 1)],
                               start=(k == 0), stop=(k == KT - 1))
                        nc.vector.tensor_copy(qkt[m][:, 512 * j:512 * (j + 1)],
                                              ps[:])

                def emit_v(i):
                    for c0, w in ((0, 512), (512, 256)):
                        ps = psY.tile([128, 512], f32, name="v_ps", tag="py")
                        for k in range(KT):
                            mm(ps[:, 0:w], xt[k][:, 128 * i:128 * (i + 1)],
                               wv[k][:, c0:c0 + w],
                               start=(k == 0), stop=(k == KT - 1))
                        # scatter heads: even -> cols 0:64, odd -> cols 65:129
                        # within each 130-wide pair block
                        v_view = ps[:, 0:w].rearrange("p (a t c) -> p a t c",
                                                      t=2, c=64)
                        pa0 = c0 // 128
                        npair = w // 128
                        nc.vector.tensor_copy(
                            vbuf[i][:, pa0:pa0 + npair, 0:64], v_view[:, :, 0, :])
                        nc.vector.tensor_copy(
                            vbuf[i][:, pa0:pa0 + npair, 65:129], v_view[:, :, 1, :])

                # head pairs 0/1 first so attention starts while the rest
                # of the QKV projection still runs; remaining Q/K tiles are
                # emitted interleaved between attention pairs (emission order
                # drives scheduler priority).
                for i in range(NT):
                    emit_v(i)
                for m in (0, 6, 1, 7):
                    emit_qk(m)

                # remaining Q/K half-groups, injected in small chunks inside
                # the attention loops (their own psum pool keeps them off the
                # score-matmul slot chain)
                # ---- attention, j-outer so P@V psum is one bank per head ----
                for t in range(PAIRS):
                    if t + 2 < PAIRS:
                        emit_qk(t + 2)
                        emit_qk(PAIRS + t + 2)
                    qt, kt = qkt[t], qkt[PAIRS + t]
                    aot = qa.tile([128, N], fm, name=f"aot{t}", tag="qa")
                    if t == 0:
                        aot_all = []
                    aot_all.append(aot)
                    for j in range(2):
                        pv_ps = [psY.tile([65, 512], f32, name=f"pv{h}", tag="py")
                                 for h in range(2)]
                        for i in range(NT):
                            stexp = pB.tile([128, 2, 512], fm, name="stexp",
                                            tag="stexp", bufs=4)
                            s_ps = psS.tile([128, 1024], f32, name="s_ps",
                                            tag="ps")
                            for h in range(2):
                                # S^T[m, n] = sum_d K^T[d, m] Q^T[d, n]; h0/h1
                                # use distinct PE row groups (base partition
                                # 0 / 64) and run concurrently.
                                mm(s_ps[:, 512 * h:512 * (h + 1)],
                                   kt[64 * h:64 * (h + 1), 128 * i:128 * (i + 1)],
                                   qt[64 * h:64 * (h + 1), 512 * j:512 * (j + 1)],
                                   start=True, stop=True)
                            # exp(S^T / 8) for both heads, PSUM -> SBUF f32r
                            nc.scalar.activation(
                                stexp[:, :, :],
                                s_ps[:].rearrange("p (h n) -> p h n", h=2),
                                Exp, scale=SCALE)
                            for h in range(2):
                                # rows 0:64 = (P~ @ V)^T, row 64 = denominator
                                mm(pv_ps[h][:],
                                   vbuf[i][:, t, 65 * h:65 * (h + 1)],
                                   stexp[:, h, :],
                                   start=(i == 0), stop=(i == NT - 1))

                        # normalization, phase-ordered so no DVE op ever
                        # head-of-line-blocks the next pair's PSUM release:
                        # copies free the P@V banks immediately; the
                        # DMA-latency-bound multiplies run last.
                        stages = []
                        for h in range(2):
                            stage = pB.tile([65, 512], f32, name="stage",
                                            tag="stage")
                            nc.vector.tensor_copy(stage[:], pv_ps[h][:])
                            stages.append(stage)
                        dens = []
                        for h in range(2):
                            # [1, 512] DVE reciprocal is FD-bound (~3us); DMA
                            # the denominator row into [128, 4] first where
                            # the same op is ~130ns.
                            den_t = pB.tile([128, 4], f32, name="den_t",
                                            tag="den_t")
                            nc.sync.dma_start(den_t[:], stages[h][64:65, :])
                            dens.append(den_t)
                        rbs = []
                        for h in range(2):
                            nc.vector.reciprocal(dens[h][:], dens[h][:])
                            dr2 = pDr.tile([1, 512], f32, name="dr2", tag="dr2")
                            nc.sync.dma_start(
                                dr2[:].rearrange("p (a b) -> (p a) b", a=128),
                                dens[h][:])
                            # partition-broadcast of the reciprocal row: SBUF
                            # APs can't have zero partition step, so broadcast
                            # from DRAM.
                            rb = pB.tile([64, 512], f32, name="rb", tag="rb")
                            nc.sync.dma_start(rb[:], dr2[:].to_broadcast((64, 512)))
                            rbs.append(rb)
                        for h in range(2):
                            if h == 0:
                                nc.vector.tensor_mul(
                                    aot[0:64, 512 * j:512 * (j + 1)],
                                    stages[0][0:64, :], rbs[0][:])
                            else:
                                tmp = pB.tile([64, 512], fm, name="tmp1",
                                              tag="tmp1")
                                nc.vector.tensor_mul(tmp[:], stages[1][0:64, :],
                                                     rbs[1][:])
                                # DVE lanes cannot shift partitions; DMA moves
                                # the odd head into partitions 64:128.
                                nc.sync.dma_start(
                                    aot[64:128, 512 * j:512 * (j + 1)], tmp[:])

            # ---- output projection: y = attn_out^T.T @ w_proj^T + b ----
            # (opened after phase A closes so wp/bias reuse xt/wqk space)
            with tc.tile_pool(name="proj", bufs=1) as pC:
                wp = [pC.tile([128, C], fm, name=f"wp{k}", tag=f"wp{k}")
                      for k in range(KT)]
                bias_t = pC.tile([128, C], f32, name="bias_t", tag="bias_t")
                for k in range(KT):
                    nc.sync.dma_start(wp[k][:], wprojT_d[128 * k:128 * (k + 1), :])
                nc.sync.dma_start(bias_t[:], bias_d[:])

                for i in range(NT):
                    yt = pB.tile([128, C], f32, name="yt", tag="yt")
                    for c0 in (0, 384):
                        # alternate the two attention psum pools so four
                        # k-accumulation groups can be in flight
                        if (2 * i + c0 // 384) % 2 == 0:
                            pp = psS.tile([128, 384], f32, name="pp", tag="ps")
                        else:
                            pp = psY.tile([128, 384], f32, name="pp", tag="py")
                        for k in range(KT):
                            mm(pp[:, 0:384],
                               aot_all[k][:, 128 * i:128 * (i + 1)],
                               wp[k][:, c0:c0 + 384],
                               start=(k == 0), stop=(k == KT - 1))
                        nc.vector.tensor_add(yt[:, c0:c0 + 384], pp[:, 0:384],
                                             bias_t[:, c0:c0 + 384])
                    nc.sync.dma_start(y_d[128 * i:128 * (i + 1), :], yt[:])

    nc.compile()
    return nc


def round_f32r(a):
    """Round fp32 to the FP32r grid (11 explicit mantissa bits, RNE) --
    what the PE reads for float32r matmuls."""
    a = np.ascontiguousarray(a, dtype=np.float32)
    b = a.view(np.uint32)
    r = (b + np.uint32(0x7FF) + ((b >> np.uint32(12)) & np.uint32(1))) \
        & np.uint32(0xFFFFF000)
    return r.view(np.float32)


def make_in_maps(x, w_qkv, w_proj, b_proj):
    wqkvT = round_f32r(np.asarray(w_qkv, dtype=np.float32).T)
    wprojT = round_f32r(np.asarray(w_proj, dtype=np.float32).T)
    bias_rep = np.ascontiguousarray(
        np.broadcast_to(np.asarray(b_proj, dtype=np.float32), (128, C)))
    x = np.asarray(x, dtype=np.float32)
    return [
        {
            "xT": round_f32r(x[b].T),
            "wqkvT": wqkvT,
            "wprojT": wprojT,
            "bias_rep": bias_rep,
        }
        for b in range(B)
    ]


def kernel(x, w_qkv, w_proj, b_proj):
    from concourse.bass_utils import run_bass_kernel_spmd

    if "nc" not in _CACHE:
        _CACHE["nc"] = build_program()
    nc = _CACHE["nc"]

    in_maps = make_in_maps(x, w_qkv, w_proj, b_proj)
    res = run_bass_kernel_spmd(nc, in_maps, core_ids=list(range(B)))
    out = np.stack([res.results[b]["y"] for b in range(B)], axis=0)
    return out.astype(np.float32)



# revision 5
# speedup vs baseline: 1.0893x; 1.0893x over previous
"""Multi-head attention (B=8, N=1024, C=768, H=12) on 8 Trainium2 NeuronCores.

Sharding: data-parallel, one batch element per core. Each core computes the
full attention block for its batch: QKV projection, per-head softmax(QK^T/8)V,
and the output projection, entirely on-chip (SBUF/PSUM).

Layout strategy (chosen so no on-device transposes are needed):
  - host passes x^T [C, N], w_qkv^T [C, 3C], w_proj^T [C, C], bias replicated
    to [128, C].
  - Q, K are produced transposed ([d, n], head-dim on partitions) by the QKV
    matmul; V is produced in natural [n, d] layout by swapping lhsT/rhs.
  - scores are computed transposed (S^T[m, n] = K Q^T) so that exp(S^T) can be
    consumed directly as the moving operand of the P@V matmul.
  - V tiles carry an appended ones-column, so the P@V matmul's 65th output row
    is the softmax denominator (row-sum of exp scores) for free.
  - normalization multiplies by a reciprocal row broadcast across partitions
    via a DRAM-bounced DMA (SBUF APs cannot partition-broadcast).

Matmul operands use dtype float32r: single-pass PE streaming (1 column/cycle,
4x faster than float32's two-pass LOW/HIGH emulation) with 11 explicit
mantissa bits. Producers round on write; DRAM inputs are pre-rounded on host.

Scheduling: attention for head pair t overlaps the remaining QKV projection
work. All PSUM users run on half-size (single-bank) accumulation groups so
the 8 banks split 2+2 (QK/V projection) + 2+2 (scores / P@V); attn_out^T
tiles reuse the SBUF slots of dead Q^T tiles so everything fits in 192KB.
"""

import sys

import numpy as np

if "/opt/trn_rl_repo" not in sys.path:
    sys.path.insert(0, "/opt/trn_rl_repo")

B = 8
N = 1024
C = 768
H = 12
D = 64
SCALE = D ** -0.5
KT = C // 128           # 6 contraction tiles over channels
MT_QK = 2 * C // 128    # 12 output tiles for Q and K (o in [0, 1536))
NT = N // 128           # 8 token tiles
PAIRS = H // 2          # 6 head pairs

_CACHE = {}


def build_program(fast=True, use_bf16=True):
    import concourse.bacc as bacc
    import concourse.mybir as mybir
    import concourse.tile as tile

    f32 = mybir.dt.float32
    f32r = mybir.dt.float32r
    Exp = mybir.ActivationFunctionType.Exp
    if use_bf16:
        fm = mybir.dt.bfloat16
    else:
        fm = f32r if fast else f32

    nc = bacc.Bacc("TRN2", target_bir_lowering=False, debug=False)

    xT_d = nc.dram_tensor("xT", [C, N], fm, kind="ExternalInput")
    wqkvT_d = nc.dram_tensor("wqkvT", [C, 3 * C], fm, kind="ExternalInput")
    wprojT_d = nc.dram_tensor("wprojT", [C, C], fm, kind="ExternalInput")
    bias_d = nc.dram_tensor("bias_rep", [128, C], f32, kind="ExternalInput")
    y_d = nc.dram_tensor("y", [N, C], f32, kind="ExternalOutput")

    mm = nc.tensor.matmul

    with tile.TileContext(nc) as tc:
        # qkt/aot share one 12-slot tag: each aot[t] lands in the slot of a
        # Q^T/K^T tile that died right before it (pair t's score matmuls).
        with tc.tile_pool(name="pers", bufs=1) as pers, \
             tc.tile_pool(name="qa", bufs=13) as qa, \
             tc.tile_pool(name="cyc", bufs=2) as pB, \
             tc.tile_pool(name="dramb", bufs=2, space="DRAM") as pDr, \
             tc.tile_pool(name="ps_s", bufs=3, space="PSUM") as psS, \
             tc.tile_pool(name="ps_y", bufs=2, space="PSUM") as psY:
            # Q^T,K^T tiles [d, n]: tile m holds heads 2m (parts 0:64) and
            # 2m+1 (parts 64:128); m 0..5 = Q, 6..11 = K.
            qkt = [qa.tile([128, N], fm, name=f"qkt{m}", tag="qa")
                   for m in range(MT_QK)]
            # V tiles [n-tile, pair, 130]: per pair block [V_h0 |1| V_h1 |1];
            # ones cols at 64 and 129 feed the denominator row of P@V.
            vbuf = [pers.tile([128, PAIRS, 130], fm, name=f"vbuf{i}", tag=f"vbuf{i}")
                    for i in range(NT)]

            with tc.tile_pool(name="phA", bufs=1) as pA:
                xt = [pA.tile([128, N], fm, name=f"xt{k}", tag=f"xt{k}")
                      for k in range(KT)]
                wqk = [pA.tile([128, 2 * C], fm, name=f"wqk{k}", tag=f"wqk{k}")
                       for k in range(KT)]
                wv = [pA.tile([128, C], fm, name=f"wv{k}", tag=f"wv{k}")
                      for k in range(KT)]
                for k in range(KT):
                    nc.sync.dma_start(xt[k][:], xT_d[128 * k:128 * (k + 1), :])
                for k in range(KT):
                    nc.sync.dma_start(wv[k][:],
                                      wqkvT_d[128 * k:128 * (k + 1), 2 * C:3 * C])
                for k in range(KT):
                    nc.sync.dma_start(wqk[k][:],
                                      wqkvT_d[128 * k:128 * (k + 1), 0:2 * C])
                for i in range(NT):
                    ones_ap = vbuf[i].rearrange("p a (t c) -> p a t c", c=65)[:, :, :, 64]
                    if fm == mybir.dt.float32r:
                        nc.vector.memset(ones_ap.bitcast(f32), 1.0)
                    else:
                        nc.vector.memset(ones_ap, 1.0)


                # ---- QKV projection, single-bank accumulation groups ----
                def emit_qk(m):
                    for j in range(2):
                        ps = psS.tile([128, 512], f32, name="qk_ps", tag="ps")
                        for k in range(KT):
                            mm(ps[:], wqk[k][:, 128 * m:128 * (m + 1)],
                               xt[k][:, 512 * j:512 * (j + 1)],
                               start=(k == 0), stop=(k == KT - 1))
                        nc.vector.tensor_copy(qkt[m][:, 512 * j:512 * (j + 1)],
                                              ps[:])

                def emit_v(i):
                    for c0, w in ((0, 512), (512, 256)):
                        ps = psY.tile([128, 512], f32, name="v_ps", tag="py")
                        for k in range(KT):
                            mm(ps[:, 0:w], xt[k][:, 128 * i:128 * (i + 1)],
                               wv[k][:, c0:c0 + w],
                               start=(k == 0), stop=(k == KT - 1))
                        # scatter heads: even -> cols 0:64, odd -> cols 65:129
                        # within each 130-wide pair block
                        v_view = ps[:, 0:w].rearrange("p (a t c) -> p a t c",
                                                      t=2, c=64)
                        pa0 = c0 // 128
                        npair = w // 128
                        nc.vector.tensor_copy(
                            vbuf[i][:, pa0:pa0 + npair, 0:64], v_view[:, :, 0, :])
                        nc.vector.tensor_copy(
                            vbuf[i][:, pa0:pa0 + npair, 65:129], v_view[:, :, 1, :])

                # head pairs 0/1 first so attention starts while the rest
                # of the QKV projection still runs; remaining Q/K tiles are
                # emitted interleaved between attention pairs (emission order
                # drives scheduler priority).
                for i in range(NT):
                    emit_v(i)
                for m in (0, 6, 1, 7):
                    emit_qk(m)

                # remaining Q/K half-groups, injected in small chunks inside
                # the attention loops (their own psum pool keeps them off the
                # score-matmul slot chain)
                # ---- attention, j-outer so P@V psum is one bank per head ----
                for t in range(PAIRS):
                    if t + 2 < PAIRS:
                        emit_qk(t + 2)
                        emit_qk(PAIRS + t + 2)
                    qt, kt = qkt[t], qkt[PAIRS + t]
                    aot = qa.tile([128, N], fm, name=f"aot{t}", tag="qa")
                    if t == 0:
                        aot_all = []
                    aot_all.append(aot)
                    for j in range(2):
                        pv_ps = [psY.tile([65, 512], f32, name=f"pv{h}", tag="py")
                                 for h in range(2)]
                        for i in range(NT):
                            stexp = pB.tile([128, 2, 512], fm, name="stexp",
                                            tag="stexp", bufs=4)
                            s_ps = psS.tile([128, 1024], f32, name="s_ps",
                                            tag="ps")
                            for h in range(2):
                                # S^T[m, n] = sum_d K^T[d, m] Q^T[d, n]; h0/h1
                                # use distinct PE row groups (base partition
                                # 0 / 64) and run concurrently.
                                mm(s_ps[:, 512 * h:512 * (h + 1)],
                                   kt[64 * h:64 * (h + 1), 128 * i:128 * (i + 1)],
                                   qt[64 * h:64 * (h + 1), 512 * j:512 * (j + 1)],
                                   start=True, stop=True)
                            # exp(S^T / 8) for both heads, PSUM -> SBUF f32r
                            nc.scalar.activation(
                                stexp[:, :, :],
                                s_ps[:].rearrange("p (h n) -> p h n", h=2),
                                Exp, scale=SCALE)
                            for h in range(2):
                                # rows 0:64 = (P~ @ V)^T, row 64 = denominator
                                mm(pv_ps[h][:],
                                   vbuf[i][:, t, 65 * h:65 * (h + 1)],
                                   stexp[:, h, :],
                                   start=(i == 0), stop=(i == NT - 1))

                        # normalization, phase-ordered so no DVE op ever
                        # head-of-line-blocks the next pair's PSUM release:
                        # copies free the P@V banks immediately; the
                        # DMA-latency-bound multiplies run last.
                        stages = []
                        for h in range(2):
                            stage = pB.tile([65, 512], f32, name="stage",
                                            tag="stage")
                            nc.vector.tensor_copy(stage[:], pv_ps[h][:])
                            stages.append(stage)
                        dens = []
                        for h in range(2):
                            # [1, 512] DVE reciprocal is FD-bound (~3us); DMA
                            # the denominator row into [128, 4] first where
                            # the same op is ~130ns.
                            den_t = pB.tile([128, 4], f32, name="den_t",
                                            tag="den_t")
                            nc.sync.dma_start(den_t[:], stages[h][64:65, :])
                            dens.append(den_t)
                        rbs = []
                        for h in range(2):
                            nc.vector.reciprocal(dens[h][:], dens[h][:])
                            dr2 = pDr.tile([1, 512], f32, name="dr2", tag="dr2")
                            nc.sync.dma_start(
                                dr2[:].rearrange("p (a b) -> (p a) b", a=128),
                                dens[h][:])
                            # partition-broadcast of the reciprocal row: SBUF
                            # APs can't have zero partition step, so broadcast
                            # from DRAM.
                            rb = pB.tile([64, 512], f32, name="rb", tag="rb")
                            nc.sync.dma_start(rb[:], dr2[:].to_broadcast((64, 512)))
                            rbs.append(rb)
                        for h in range(2):
                            if h == 0:
                                nc.vector.tensor_mul(
                                    aot[0:64, 512 * j:512 * (j + 1)],
                                    stages[0][0:64, :], rbs[0][:])
                            else:
                                tmp = pB.tile([64, 512], fm, name="tmp1",
                                              tag="tmp1")
                                nc.vector.tensor_mul(tmp[:], stages[1][0:64, :],
                                                     rbs[1][:])
                                # DVE lanes cannot shift partitions; DMA moves
                                # the odd head into partitions 64:128.
                                nc.sync.dma_start(
                                    aot[64:128, 512 * j:512 * (j + 1)], tmp[:])

            # ---- output projection: y = attn_out^T.T @ w_proj^T + b ----
            # (opened after phase A closes so wp/bias reuse xt/wqk space)
            with tc.tile_pool(name="proj", bufs=1) as pC:
                wp = [pC.tile([128, C], fm, name=f"wp{k}", tag=f"wp{k}")
                      for k in range(KT)]
                bias_t = pC.tile([128, C], f32, name="bias_t", tag="bias_t")
                for k in range(KT):
                    nc.sync.dma_start(wp[k][:], wprojT_d[128 * k:128 * (k + 1), :])
                nc.sync.dma_start(bias_t[:], bias_d[:])

                for i in range(NT):
                    yt = pB.tile([128, C], f32, name="yt", tag="yt")
                    for c0 in (0, 384):
                        # alternate the two attention psum pools so four
                        # k-accumulation groups can be in flight
                        if (2 * i + c0 // 384) % 2 == 0:
                            pp = psS.tile([128, 384], f32, name="pp", tag="ps")
                        else:
                            pp = psY.tile([128, 384], f32, name="pp", tag="py")
                        for k in range(KT):
                            mm(pp[:, 0:384],
                               aot_all[k][:, 128 * i:128 * (i + 1)],
                               wp[k][:, c0:c0 + 384],
                               start=(k == 0), stop=(k == KT - 1))
                        nc.vector.tensor_add(yt[:, c0:c0 + 384], pp[:, 0:384],
                                             bias_t[:, c0:c0 + 384])
                    nc.sync.dma_start(y_d[128 * i:128 * (i + 1), :], yt[:])

    nc.compile()
    return nc


def round_f32r(a):
    """Round fp32 to the FP32r grid (11 explicit mantissa bits, RNE) --
    what the PE reads for float32r matmuls."""
    a = np.ascontiguousarray(a, dtype=np.float32)
    b = a.view(np.uint32)
    r = (b + np.uint32(0x7FF) + ((b >> np.uint32(12)) & np.uint32(1))) \
        & np.uint32(0xFFFFF000)
    return r.view(np.float32)


USE_BF16 = True


def make_in_maps(x, w_qkv, w_proj, b_proj):
    if USE_BF16:
        import ml_dtypes
        cvt = lambda a: np.ascontiguousarray(a).astype(ml_dtypes.bfloat16)
    else:
        cvt = round_f32r
    wqkvT = cvt(np.asarray(w_qkv, dtype=np.float32).T)
    wprojT = cvt(np.asarray(w_proj, dtype=np.float32).T)
    bias_rep = np.ascontiguousarray(
        np.broadcast_to(np.asarray(b_proj, dtype=np.float32), (128, C)))
    x = np.asarray(x, dtype=np.float32)
    return [
        {
            "xT": cvt(x[b].T),
            "wqkvT": wqkvT,
            "wprojT": wprojT,
            "bias_rep": bias_rep,
        }
        for b in range(B)
    ]


def kernel(x, w_qkv, w_proj, b_proj):
    from concourse.bass_utils import run_bass_kernel_spmd

    if "nc" not in _CACHE:
        _CACHE["nc"] = build_program(use_bf16=USE_BF16)
    nc = _CACHE["nc"]

    in_maps = make_in_maps(x, w_qkv, w_proj, b_proj)
    res = run_bass_kernel_spmd(nc, in_maps, core_ids=list(range(B)))
    out = np.stack([res.results[b]["y"] for b in range(B)], axis=0)
    return out.astype(np.float32)



# revision 6
# speedup vs baseline: 1.1213x; 1.0293x over previous
"""Multi-head attention (B=8, N=1024, C=768, H=12) on 8 Trainium2 NeuronCores.

Sharding: data-parallel, one batch element per core. Each core computes the
full attention block for its batch: QKV projection, per-head softmax(QK^T/8)V,
and the output projection, entirely on-chip (SBUF/PSUM).

Layout strategy (chosen so no on-device transposes are needed):
  - host passes x^T [C, N], w_qkv^T [C, 3C], w_proj^T [C, C], bias replicated
    to [128, C].
  - Q, K are produced transposed ([d, n], head-dim on partitions) by the QKV
    matmul; V is produced in natural [n, d] layout by swapping lhsT/rhs.
  - scores are computed transposed (S^T[m, n] = K Q^T) so that exp(S^T) can be
    consumed directly as the moving operand of the P@V matmul.
  - V tiles carry an appended ones-column, so the P@V matmul's 65th output row
    is the softmax denominator (row-sum of exp scores) for free.
  - normalization multiplies by a reciprocal row broadcast across partitions
    via a DRAM-bounced DMA (SBUF APs cannot partition-broadcast).

Matmul operands are bf16 (single-pass PE streaming, FWL-eligible weight
loads); PSUM accumulation stays fp32.

Scheduling (v2): the exp stream on the ACT engine is the long pole
(~132us of exp for 12.6M scores), so the schedule is built around keeping
ACT busy from ~7us on:
  - Q/K for head pair 0 are projected first; its score matmuls + exps for
    both query halves are emitted before anything else so ACT starts early.
  - V projection, later pairs' Q/K projections, and 2/3 of the output
    projection all run inside the ACT-bound window as low-priority PE
    filler.
  - the output projection is split k-wise: pairs 0-3 are partially
    accumulated into SBUF (yacc) while pairs 4-5 still run; only the k=4,5
    sweep remains in the tail.
"""

import sys

import numpy as np

if "/opt/trn_rl_repo" not in sys.path:
    sys.path.insert(0, "/opt/trn_rl_repo")

B = 8
N = 1024
C = 768
H = 12
D = 64
SCALE = D ** -0.5
KT = C // 128            # 6 contraction tiles over channels
NT = N // 128             # 8 token tiles
PAIRS = H // 2            # 6 head pairs

_CACHE = {}


def build_program(use_bf16=True):
    import concourse.bacc as bacc
    import concourse.mybir as mybir
    import concourse.tile as tile

    f32 = mybir.dt.float32
    Exp = mybir.ActivationFunctionType.Exp
    fm = mybir.dt.bfloat16 if use_bf16 else mybir.dt.float32r

    nc = bacc.Bacc("TRN2", target_bir_lowering=False, debug=False)

    xT_d = nc.dram_tensor("xT", [C, N], fm, kind="ExternalInput")
    wqkvT_d = nc.dram_tensor("wqkvT", [C, 3 * C], fm, kind="ExternalInput")
    wprojT_d = nc.dram_tensor("wprojT", [C, C], fm, kind="ExternalInput")
    bias_d = nc.dram_tensor("bias_rep", [128, C], f32, kind="ExternalInput")
    y_d = nc.dram_tensor("y", [N, C], f32, kind="ExternalOutput")

    mm = nc.tensor.matmul

    with tile.TileContext(nc) as tc:
        with tc.tile_pool(name="pers", bufs=1) as pers, \
             tc.tile_pool(name="qa", bufs=13) as qa, \
             tc.tile_pool(name="stp", bufs=16) as stp, \
             tc.tile_pool(name="cyc", bufs=2) as pB, \
             tc.tile_pool(name="dramb", bufs=2, space="DRAM") as pDr, \
             tc.tile_pool(name="ps_s", bufs=2, space="PSUM") as psS, \
             tc.tile_pool(name="ps_y", bufs=2, space="PSUM") as psY, \
             tc.tile_pool(name="ps_p", bufs=2, space="PSUM") as psP:
            # Q^T,K^T tiles [d, n]: tile m holds heads 2m (parts 0:64) and
            # 2m+1 (parts 64:128); m 0..5 = Q, 6..11 = K. aot (attn out^T)
            # shares the 13-slot tag chain, reusing dead Q/K slots.
            qkt = [None] * (2 * PAIRS)
            # V tiles [n-tile, pair, 130]: per pair block [V_h0 |1| V_h1 |1];
            # ones cols at 64 and 129 feed the denominator row of P@V.
            vbuf = [pers.tile([128, PAIRS, 130], fm, name=f"vbuf{i}", tag=f"vbuf{i}")
                    for i in range(NT)]
            xt = [pers.tile([128, N], fm, name=f"xt{k}", tag=f"xt{k}")
                  for k in range(KT)]
            # per-(m,k) Q/K weight slices so each pair's weights DMA
            # just-in-time without multi-writer tiles
            wqkm = [[pers.tile([128, 128], fm, name=f"wq{m}_{k}", tag=f"wq{m}_{k}")
                     for k in range(KT)] for m in range(2 * PAIRS)]
            wv = [pers.tile([128, C], fm, name=f"wv{k}", tag=f"wv{k}")
                  for k in range(KT)]
            wp = [pers.tile([128, C], fm, name=f"wp{k}", tag=f"wp{k}")
                  for k in range(KT)]
            bias_t = pers.tile([128, C], f32, name="bias_t", tag="bias_t")
            yacc = [pers.tile([128, C], f32, name=f"yacc{i}", tag=f"yacc{i}")
                    for i in range(NT)]

            def dma_qk_w(m):
                o0 = 128 * m if m < PAIRS else 768 + 128 * (m - PAIRS)
                for k in range(KT):
                    nc.sync.dma_start(wqkm[m][k][:],
                                      wqkvT_d[128 * k:128 * (k + 1), o0:o0 + 128])

            for k in range(KT):
                nc.sync.dma_start(xt[k][:], xT_d[128 * k:128 * (k + 1), :])
            dma_qk_w(0)
            dma_qk_w(PAIRS)
            for k in range(KT):
                nc.sync.dma_start(wv[k][:],
                                  wqkvT_d[128 * k:128 * (k + 1), 2 * C:3 * C])
            for i in range(NT):
                ones_ap = vbuf[i].rearrange("p a (t c) -> p a t c", c=65)[:, :, :, 64]
                nc.vector.memset(ones_ap, 1.0)

            def emit_qk(m):
                t_ = qa.tile([128, N], fm, name=f"qkt{m}", tag="qa")
                qkt[m] = t_
                for j in range(2):
                    ps = psS.tile([128, 512], f32, name="qk_ps", tag="ps")
                    for k in range(KT):
                        mm(ps[:], wqkm[m][k][:],
                           xt[k][:, 512 * j:512 * (j + 1)],
                           start=(k == 0), stop=(k == KT - 1))
                    nc.vector.tensor_copy(t_[:, 512 * j:512 * (j + 1)], ps[:])

            def emit_v(i):
                for c0, w in ((0, 512), (512, 256)):
                    ps = psY.tile([128, 512], f32, name="v_ps", tag="py")
                    for k in range(KT):
                        mm(ps[:, 0:w], xt[k][:, 128 * i:128 * (i + 1)],
                           wv[k][:, c0:c0 + w],
                           start=(k == 0), stop=(k == KT - 1))
                    # scatter heads: even -> cols 0:64, odd -> cols 65:129
                    # within each 130-wide pair block
                    v_view = ps[:, 0:w].rearrange("p (a t c) -> p a t c",
                                                  t=2, c=64)
                    pa0 = c0 // 128
                    npair = w // 128
                    nc.vector.tensor_copy(
                        vbuf[i][:, pa0:pa0 + npair, 0:64], v_view[:, :, 0, :])
                    nc.vector.tensor_copy(
                        vbuf[i][:, pa0:pa0 + npair, 65:129], v_view[:, :, 1, :])

            # scores + exp for (pair t, query half j): feeds the ACT stream
            def emit_scores(t, j, stexps):
                qt, kt = qkt[t], qkt[PAIRS + t]
                for i in range(NT):
                    stexp = stp.tile([128, 2, 512], fm, name="stexp",
                                     tag="stexp")
                    s_ps = psS.tile([128, 1024], f32, name="s_ps", tag="ps")
                    for h in range(2):
                        # S^T[m, n] = sum_d K^T[d, m] Q^T[d, n]; h0/h1 use
                        # distinct PE row groups (base partition 0 / 64).
                        mm(s_ps[:, 512 * h:512 * (h + 1)],
                           kt[64 * h:64 * (h + 1), 128 * i:128 * (i + 1)],
                           qt[64 * h:64 * (h + 1), 512 * j:512 * (j + 1)],
                           start=True, stop=True)
                    # exp(S^T / 8) for both heads, PSUM -> SBUF bf16
                    nc.scalar.activation(
                        stexp[:, :, :],
                        s_ps[:].rearrange("p (h n) -> p h n", h=2),
                        Exp, scale=SCALE)
                    stexps.append(stexp)

            # P@V + normalization for (pair t, query half j)
            def emit_pv(t, j, stexps, aot):
                pv_ps = [psY.tile([65, 512], f32, name=f"pv{h}", tag="py")
                         for h in range(2)]
                for i in range(NT):
                    stexp = stexps[i]
                    for h in range(2):
                        # rows 0:64 = (P~ @ V)^T, row 64 = denominator
                        mm(pv_ps[h][:],
                           vbuf[i][:, t, 65 * h:65 * (h + 1)],
                           stexp[:, h, :],
                           start=(i == 0), stop=(i == NT - 1))

                # normalization, phase-ordered so no DVE op ever
                # head-of-line-blocks the next group's PSUM release
                stages = []
                for h in range(2):
                    stage = pB.tile([65, 512], f32, name="stage", tag="stage")
                    nc.vector.tensor_copy(stage[:], pv_ps[h][:])
                    stages.append(stage)
                dens = []
                for h in range(2):
                    # [1, 512] DVE reciprocal is FD-bound (~3us); DMA the
                    # denominator row into [128, 4] first where it's ~130ns.
                    den_t = pB.tile([128, 4], f32, name="den_t", tag="den_t")
                    nc.sync.dma_start(den_t[:], stages[h][64:65, :])
                    dens.append(den_t)
                rbs = []
                for h in range(2):
                    nc.vector.reciprocal(dens[h][:], dens[h][:])
                    dr2 = pDr.tile([1, 512], f32, name="dr2", tag="dr2")
                    nc.sync.dma_start(
                        dr2[:].rearrange("p (a b) -> (p a) b", a=128),
                        dens[h][:])
                    # partition-broadcast of the reciprocal row: SBUF APs
                    # can't have zero partition step, so broadcast from DRAM.
                    rb = pB.tile([64, 512], f32, name="rb", tag="rb")
                    nc.sync.dma_start(rb[:], dr2[:].to_broadcast((64, 512)))
                    rbs.append(rb)
                for h in range(2):
                    if h == 0:
                        nc.vector.tensor_mul(
                            aot[0:64, 512 * j:512 * (j + 1)],
                            stages[0][0:64, :], rbs[0][:])
                    else:
                        tmp = pB.tile([64, 512], fm, name="tmp1", tag="tmp1")
                        nc.vector.tensor_mul(tmp[:], stages[1][0:64, :],
                                             rbs[1][:])
                        # DVE lanes cannot shift partitions; DMA moves the
                        # odd head into partitions 64:128.
                        nc.sync.dma_start(
                            aot[64:128, 512 * j:512 * (j + 1)], tmp[:])

            # output projection sweep over pairs k0..k1 for token tiles isl;
            # k<4 accumulates bias+partials into yacc, k>=4 finishes into yt
            def emit_proj(k0, k1, isl, aot_all):
                for i in isl:
                    if k0 == 0:
                        dst = yacc[i]
                    else:
                        dst = pB.tile([128, C], f32, name="yt", tag="yt")
                    for c0 in (0, 384):
                        pp = psP.tile([128, 384], f32, name="pp", tag="pp")
                        for k in range(k0, k1):
                            mm(pp[:],
                               aot_all[k][:, 128 * i:128 * (i + 1)],
                               wp[k][:, c0:c0 + 384],
                               start=(k == k0), stop=(k == k1 - 1))
                        if k0 == 0:
                            nc.vector.tensor_add(dst[:, c0:c0 + 384], pp[:],
                                                 bias_t[:, c0:c0 + 384])
                        else:
                            nc.vector.tensor_add(dst[:, c0:c0 + 384], pp[:],
                                                 yacc[i][:, c0:c0 + 384])
                    if k0 != 0:
                        nc.sync.dma_start(y_d[128 * i:128 * (i + 1), :], dst[:])

            # ---- emission schedule ----
            emit_qk(0)
            emit_qk(PAIRS)
            aot_all = []
            st00, st01 = [], []
            emit_scores(0, 0, st00)      # ACT stream starts here (~7us in)
            emit_scores(0, 1, st01)
            dma_qk_w(1)
            dma_qk_w(PAIRS + 1)
            emit_qk(1)
            emit_qk(PAIRS + 1)
            for k in range(KT):          # proj weights early, low priority
                nc.sync.dma_start(wp[k][:], wprojT_d[128 * k:128 * (k + 1), :])
            nc.sync.dma_start(bias_t[:], bias_d[:])
            for i in range(NT):
                emit_v(i)
            aot0 = qa.tile([128, N], fm, name="aot0", tag="qa")
            aot_all.append(aot0)
            emit_pv(0, 0, st00, aot0)
            emit_pv(0, 1, st01, aot0)

            for t in range(1, PAIRS):
                aot = qa.tile([128, N], fm, name=f"aot{t}", tag="qa")
                aot_all.append(aot)
                stj0, stj1 = [], []
                emit_scores(t, 0, stj0)
                if t + 1 < PAIRS:
                    dma_qk_w(t + 1)
                    dma_qk_w(PAIRS + t + 1)
                    emit_qk(t + 1)
                    emit_qk(PAIRS + t + 1)
                emit_pv(t, 0, stj0, aot)
                emit_scores(t, 1, stj1)
                emit_pv(t, 1, stj1, aot)
                if t == 3:
                    emit_proj(0, 4, range(0, 4), aot_all)
                elif t == 4:
                    emit_proj(0, 4, range(4, NT), aot_all)
            emit_proj(4, KT, range(NT), aot_all)

    nc.compile()
    return nc


def round_f32r(a):
    """Round fp32 to the FP32r grid (11 explicit mantissa bits, RNE)."""
    a = np.ascontiguousarray(a, dtype=np.float32)
    b = a.view(np.uint32)
    r = (b + np.uint32(0x7FF) + ((b >> np.uint32(12)) & np.uint32(1))) \
        & np.uint32(0xFFFFF000)
    return r.view(np.float32)


USE_BF16 = True


def make_in_maps(x, w_qkv, w_proj, b_proj):
    if USE_BF16:
        import ml_dtypes
        cvt = lambda a: np.ascontiguousarray(a).astype(ml_dtypes.bfloat16)
    else:
        cvt = round_f32r
    wqkvT = cvt(np.asarray(w_qkv, dtype=np.float32).T)
    wprojT = cvt(np.asarray(w_proj, dtype=np.float32).T)
    bias_rep = np.ascontiguousarray(
        np.broadcast_to(np.asarray(b_proj, dtype=np.float32), (128, C)))
    x = np.asarray(x, dtype=np.float32)
    return [
        {
            "xT": cvt(x[b].T),
            "wqkvT": wqkvT,
            "wprojT": wprojT,
            "bias_rep": bias_rep,
        }
        for b in range(B)
    ]


def kernel(x, w_qkv, w_proj, b_proj):
    from concourse.bass_utils import run_bass_kernel_spmd

    if "nc" not in _CACHE:
        _CACHE["nc"] = build_program(use_bf16=USE_BF16)
    nc = _CACHE["nc"]

    in_maps = make_in_maps(x, w_qkv, w_proj, b_proj)
    res = run_bass_kernel_spmd(nc, in_maps, core_ids=list(range(B)))
    out = np.stack([res.results[b]["y"] for b in range(B)], axis=0)
    return out.astype(np.float32)
